# revision 5
# baseline (speedup 1.0000x reference)
"""Trainium2 Bass kernel for nn_ASTDecoder (banded-adjacency GCN stack).

Structure exploited (graph is fixed: nodes i<->i+1, i<->i+2 within each
256-node graph, plus self loops; initial node features are identical for
all nodes of a graph):
  * deg = [3,4,5,...,5,4,3]; interior rows of the normalized adjacency
    sum to exactly 1, so nodes far from the graph boundary keep a single
    per-graph value through all 3 conv layers.  After 3 layers only nodes
    0..7 and 248..255 differ from the per-graph constant; the two ends
    are mirror images of each other.
  * Per core (256 graphs): compute the per-graph constant chain and a
    narrow boundary strip exactly (strip rows that would equal the
    constant are taken from the chain, so layer l only computes rows
    0..2l+1), then write the output: 240 interior rows per graph are a
    replicated 512B row (the bulk of the 32MB per-core output traffic),
    16 edge rows come from the strip.

Program order is chosen so the interior-output path (chain -> W_out ->
transpose -> replicate -> 30MB of DMA) is emitted before any strip work:
engine instruction order is fixed at schedule time, and the big DMAs
must start as early as possible; the strip/edge work overlaps them.
All small constants ride in one packed input tensor (one DMA instead of
seven serial ones on the head of the critical path).
"""

import os
import sys

import numpy as np

for _p in ("/opt/trn_rl_repo", "/root/.axon_site/_ro/trn_rl_repo"):
    if os.path.isdir(_p) and _p not in sys.path:
        sys.path.insert(0, _p)

from concourse import bacc, mybir, tile
from concourse.bass import broadcast_tensor_aps
from concourse.bass_utils import run_bass_kernel_spmd

B, E, H, O, N = 2048, 256, 64, 128, 256
CORES = 8
G = B // CORES  # graphs per core
S = 16          # boundary strip width (nodes that can differ, per side)
R = 48          # interior replication factor (240 = 5 * 48)
CP = 581        # packed-constants width: wemb 128 | wl 192 | wout 128 | biash 4 | bout 1 | ident 128
AF = mybir.ActivationFunctionType
ALU = mybir.AluOpType
DT = mybir.dt.float32


def _consts():
    deg = np.full(N, 5.0, np.float32)
    deg[[0, -1]] = 3.0
    deg[[1, -2]] = 4.0
    dinv = (1.0 / np.sqrt(deg)).astype(np.float32)
    q = np.float32(dinv[128] * dinv[128])
    c_int = np.float32(np.float32(5.0) * q)
    Bm = np.zeros((S, S), np.float32)
    for i in range(S):
        for j in range(S):
            if abs(i - j) <= 2:
                Bm[i, j] = np.float32(dinv[i] * dinv[j])
    # diagonal coefficient table: coefd[di, i] = Bm[i, i + di - 2]
    coefd = np.zeros((5, S), np.float32)
    for di in range(5):
        d = di - 2
        for i in range(S):
            j = i + d
            if 0 <= j < S:
                coefd[di, i] = Bm[i, j]
    # layer-0 row sums (strip rows 0..3): output = relu(rowsum * g0 + b)
    rsum = np.array([Bm[i, :].sum(dtype=np.float64) for i in range(4)], np.float32)
    return q, c_int, Bm, coefd, rsum


_Q, _CINT, _BM, _COEFD, _RSUM = _consts()
_NC = None


def _build():
    nc = bacc.Bacc("TRN2", target_bir_lowering=False, debug=False, num_devices=CORES)
    emb = nc.dram_tensor("emb", [G, E], DT, kind="ExternalInput")
    cpack = nc.dram_tensor("cpack", [128, CP], DT, kind="ExternalInput")
    coefd = nc.dram_tensor("coefd", [H, 5, S, 1], DT, kind="ExternalInput")
    out = nc.dram_tensor("out", [G, N, O], DT, kind="ExternalOutput")

    with tile.TileContext(nc) as tc:
        with (
            tc.tile_pool(name="const", bufs=1) as constp,
            tc.tile_pool(name="embp", bufs=1) as embp,
            tc.tile_pool(name="stripp", bufs=1) as stripp,
            tc.tile_pool(name="scr", bufs=1) as scr,
            tc.tile_pool(name="chain", bufs=1) as chainp,
            tc.tile_pool(name="outp", bufs=1) as outp,
            tc.tile_pool(name="ps", bufs=2, space="PSUM") as ps,
        ):
            # ---- inputs: emb halves first, then packed constants
            emb_sb = embp.tile([128, 2, E], DT)
            cp_sb = constp.tile([128, CP], DT)
            nc.sync.dma_start(emb_sb[:, 0, :], emb.ap()[0:128, :])
            nc.sync.dma_start(emb_sb[:, 1, :], emb.ap()[128:256, :])
            nc.sync.dma_start(cp_sb[:], cpack.ap())
            coefd_sb = constp.tile([H, 5, S, 1], DT)
            nc.scalar.dma_start(coefd_sb[:], coefd.ap())

            wemb = lambda ec: cp_sb[:, ec * 64:(ec + 1) * 64]
            wl = lambda l: cp_sb[0:64, 128 + l * 64:128 + (l + 1) * 64]
            wout_sb = cp_sb[0:64, 320:448]
            bias = lambda k: cp_sb[0:64, 448 + k:449 + k]
            bout_sb = cp_sb[0:128, 452:453]
            ident_sb = cp_sb[:, 453:581]

            # ---- embedding PE transpose to [e, g]
            embT = embp.tile([128, 2, G], DT)
            for ec in range(2):
                for gc in range(2):
                    tp = ps.tile([128, 128], DT, tag="tr", bufs=4)
                    nc.tensor.transpose(
                        tp[:], emb_sb[:, gc, ec * 128:(ec + 1) * 128], ident_sb
                    )
                    nc.vector.tensor_copy(embT[:, ec, gc * 128:(gc + 1) * 128], tp[:])

            # ---- init = W_emb.T @ emb.T + b_emb   (layout [h, g])
            ip = ps.tile([H, G], DT, tag="mm")
            nc.tensor.matmul(ip[:], wemb(0), embT[:, 0, :], start=True, stop=False)
            nc.tensor.matmul(ip[:], wemb(1), embT[:, 1, :], start=False, stop=True)
            v = chainp.tile([H, G], DT, tag="v0")
            nc.scalar.activation(v[:], ip[:], AF.Identity, bias=bias(0), scale=1.0)

            # ---- per-graph constant chain; g_l spilled to SBUF for the strip
            g_sb = []
            vs = [v]
            for l in range(3):
                gp = ps.tile([H, G], DT, tag="mm")
                nc.tensor.matmul(gp[:], wl(l), vs[-1][:], start=True, stop=True)
                vn = chainp.tile([H, G], DT, tag=f"v{l + 1}")
                nc.scalar.activation(vn[:], gp[:], AF.Relu, bias=bias(l + 1), scale=float(_CINT))
                gs_ = chainp.tile([H, G], DT, tag=f"g{l}s")
                nc.scalar.copy(gs_[:], gp[:])
                vs.append(vn)
                g_sb.append(gs_)
            v3 = vs[3]

            # ---- interior output row per graph: oi[o, g] = W_out.T @ v3 + b_out
            op_ = ps.tile([O, G], DT, tag="mm")
            nc.tensor.matmul(op_[:], wout_sb, v3[:], start=True, stop=True)
            oi = outp.tile([O, G], DT)
            nc.scalar.activation(oi[:], op_[:], AF.Identity, bias=bout_sb, scale=1.0)
            intg = outp.tile([128, 2, O], DT)
            rep = outp.tile([128, 2, R * O], DT)
            for gc in range(2):
                tp = ps.tile([128, 128], DT, tag="tr", bufs=4)
                nc.tensor.transpose(tp[:], oi[:, gc * 128:(gc + 1) * 128], ident_sb)
                nc.vector.tensor_copy(intg[:, gc, :], tp[:])
                # replicate each graph's interior row R times -> 5 big DMAs
                nc.vector.tensor_copy(rep[:, gc, 0:O], intg[:, gc, :])
                w = O
                while w < R * O:
                    cw = min(w, R * O - w)
                    nc.vector.tensor_copy(rep[:, gc, w:w + cw], rep[:, gc, 0:cw])
                    w += cw
                for k in range(240 // R):
                    dst = out.ap()[gc * 128:(gc + 1) * 128, 8 + k * R:8 + (k + 1) * R, :]
                    nc.sync.dma_start(out=dst, in_=rep[:, gc, :])

            # ---- boundary strip (overlaps the interior DMAs)
            def diag_combine(pre, hs, nrows):
                """pre[:, i, :] = sum_d coefd[d, i] * hs[:, i+d-2, :], rows 0..nrows."""
                tmp = scr.tile([H, 8, G], DT, tag="tmp")
                for di in (2, 0, 1, 3, 4):  # center diagonal first: full-range init
                    d = di - 2
                    lo = max(0, -d)
                    cnt = nrows - lo
                    w_in = hs[:, lo + d:lo + d + cnt, :]
                    cf = coefd_sb[:, di, lo:lo + cnt, :]
                    cfb, _ = broadcast_tensor_aps(cf, w_in)
                    if di == 2:
                        nc.vector.tensor_tensor(pre[:, lo:nrows, :], w_in, cfb, ALU.mult)
                    else:
                        nc.vector.tensor_tensor(tmp[:, lo:nrows, :], w_in, cfb, ALU.mult)
                        nc.vector.tensor_tensor(
                            pre[:, lo:nrows, :], pre[:, lo:nrows, :],
                            tmp[:, lo:nrows, :], ALU.add,
                        )

            # layer 0: strip rows 0..3 are just scaled chain values
            strip1 = stripp.tile([H, 4, G], DT, tag="s1")
            for i in range(4):
                nc.scalar.activation(
                    strip1[:, i, :], g_sb[0][:], AF.Relu, bias=bias(1), scale=float(_RSUM[i])
                )

            # layer 1: hs rows 0..3 real, rows 4..7 = g1; combine rows 0..5
            hs1 = stripp.tile([H, 8, G], DT, tag="hs1")
            for c in range(2):
                hp = ps.tile([H, 512], DT, tag="mm")
                nc.tensor.matmul(
                    hp[:], wl(1), strip1[:, 2 * c:2 * c + 2, :], start=True, stop=True
                )
                nc.scalar.copy(hs1[:, 2 * c:2 * c + 2, :], hp[:])
            nc.vector.tensor_copy(hs1[:, 4, :], g_sb[1][:])
            nc.vector.tensor_copy(hs1[:, 5, :], hs1[:, 4, :])
            nc.vector.tensor_copy(hs1[:, 6:8, :], hs1[:, 4:6, :])
            pre1 = scr.tile([H, 8, G], DT, tag="pre")
            diag_combine(pre1, hs1, 6)
            strip2 = stripp.tile([H, 6, G], DT, tag="s2")
            nc.scalar.activation(strip2[:], pre1[:, 0:6, :], AF.Relu, bias=bias(2), scale=1.0)

            # layer 2: hs rows 0..5 real, rows 6..9 = g2; combine rows 0..7
            hs2 = stripp.tile([H, 10, G], DT, tag="hs2")
            for c in range(3):
                hp = ps.tile([H, 512], DT, tag="mm")
                nc.tensor.matmul(
                    hp[:], wl(2), strip2[:, 2 * c:2 * c + 2, :], start=True, stop=True
                )
                nc.scalar.copy(hs2[:, 2 * c:2 * c + 2, :], hp[:])
            nc.vector.tensor_copy(hs2[:, 6, :], g_sb[2][:])
            nc.vector.tensor_copy(hs2[:, 7, :], hs2[:, 6, :])
            nc.vector.tensor_copy(hs2[:, 8:10, :], hs2[:, 6:8, :])
            pre2 = scr.tile([H, 8, G], DT, tag="pre")
            diag_combine(pre2, hs2, 8)
            strip3 = stripp.tile([H, 8, G], DT, tag="s3")
            nc.scalar.activation(strip3[:], pre2[:], AF.Relu, bias=bias(3), scale=1.0)

            # ---- edge rows: strip nodes 0..7 and mirrored 248..255
            edge2 = outp.tile([O, 8, G], DT)
            for c in range(4):
                ep = ps.tile([O, 512], DT, tag="mm")
                nc.tensor.matmul(
                    ep[:], wout_sb, strip3[:, 2 * c:2 * c + 2, :], start=True, stop=True
                )
                nc.scalar.activation(
                    edge2[:, 2 * c:2 * c + 2, :], ep[:], AF.Identity, bias=bout_sb, scale=1.0
                )
            for gc in range(2):
                gs = slice(gc * 128, (gc + 1) * 128)
                eL = outp.tile([128, 8, O], DT, tag=f"eL{gc}")
                eR = outp.tile([128, 8, O], DT, tag=f"eR{gc}")
                for i in range(8):
                    tp = ps.tile([128, 128], DT, tag="tr", bufs=4)
                    nc.tensor.transpose(tp[:], edge2[:, i, gs], ident_sb)
                    nc.vector.tensor_copy(eL[:, i, :], tp[:])
                    tp2 = ps.tile([128, 128], DT, tag="tr", bufs=4)
                    nc.tensor.transpose(tp2[:], edge2[:, 7 - i, gs], ident_sb)
                    nc.scalar.copy(eR[:, i, :], tp2[:])
                nc.scalar.dma_start(out.ap()[gs, 0:8, :], eL[:])
                nc.scalar.dma_start(out.ap()[gs, 248:256, :], eR[:])

    nc.compile()
    return nc


def _get_nc():
    global _NC
    if _NC is None:
        _NC = _build()
    return _NC


def _prepare_in_maps(inputs):
    f32 = lambda x: np.ascontiguousarray(np.asarray(x, dtype=np.float32))
    emb = f32(inputs["embedding"])
    w_emb = f32(inputs["W_emb"])
    b_emb = f32(inputs["b_emb"])
    conv_w = f32(inputs["conv_W"])
    conv_b = f32(inputs["conv_b"])
    w_out = f32(inputs["W_out"])
    b_out = f32(inputs["b_out"])

    cp = np.zeros((128, CP), np.float32)
    wr = w_emb.reshape(2, 128, H)
    cp[:, 0:64] = wr[0]
    cp[:, 64:128] = wr[1]
    for l in range(3):
        cp[0:64, 128 + l * 64:128 + (l + 1) * 64] = conv_w[l]
    cp[0:64, 320:448] = w_out
    cp[0:64, 448] = b_emb
    for l in range(3):
        cp[0:64, 449 + l] = conv_b[l]
    cp[0:128, 452] = b_out
    cp[:, 453:581] = np.eye(128, dtype=np.float32)

    coefd = np.ascontiguousarray(
        np.broadcast_to(_COEFD[None, :, :, None], (H, 5, S, 1)).astype(np.float32)
    )
    shared = {"cpack": cp, "coefd": coefd}
    return [dict(shared, emb=emb[c * G:(c + 1) * G]) for c in range(CORES)]


def kernel(**inputs):
    nc = _get_nc()
    in_maps = _prepare_in_maps(inputs)
    res = run_bass_kernel_spmd(nc, in_maps, core_ids=list(range(CORES)))
    return np.concatenate([r["out"] for r in res.results], axis=0)
